# revision 7
# baseline (speedup 1.0000x reference)
"""GPTQ int4 quantized linear (CaiQuantLinear) on 8 Trainium2 NeuronCores.

y = x @ dequant(qweight, scales, qzeros) + bias
  x: [8192, 4096] f32, qweight: [256, 4096] int64 (16x 4-bit packed along
  infeatures), scales: [32, 4096] f32, qzeros: [32, 256] int64 (packed along
  outfeatures), g_idx = arange(4096)//128, bias: [4096] f32 -> y: [8192, 4096] f32

Sharding: 4 token-shards x 2 outfeature-shards = 8 cores. Core c handles
tokens [2048*(c//2), +2048) and outfeatures [2048*(c%2), +2048).

Device kernel (per core): weights ship as one byte per nibble with the
4-bit code in the HIGH bits (host bit-shuffle only), so dequant is two
tensor_tensor ops: (q_u8 - 16z) * (s/16), with scale/zero rows shipped
compact (262KB) and replicated across partitions on-chip by 0-stride
partition-broadcast DMAs (writes count against the ~420GB/s DMA fabric,
HBM reads are 128x smaller).

The load phase is fabric-throughput-bound (x shard 16.8MB + first weights
+ replication writes ~ 31MB at ~420GB/s aggregate). To cover it, the first
TWO outfeature blocks are computed as one b-outer phase over 8 concurrent
PSUM groups (4 token-blocks x 2 obs), with token-blocks 0-3 shipped
k-major: the PE consumes every arriving 32KB x-slice and weight chunk
immediately and burns 112us of matmuls against the ~75us fill. Remaining
blocks run token-outer with the slim stream trickling in. Matmuls
accumulate 32 k-tiles of [128,128]x[128,256] bf16 into PSUM; evacuation
adds the bias.
"""

import sys

if "/opt/trn_rl_repo" not in sys.path:
    sys.path.insert(0, "/opt/trn_rl_repo")

import numpy as np
import ml_dtypes

import concourse.bass as bass  # noqa: F401  (registers mybir types)
import concourse.mybir as mybir
import concourse.tile as tile
from concourse import bacc
from concourse.bass_utils import run_bass_kernel_spmd

BF16 = mybir.dt.bfloat16
F32 = mybir.dt.float32
U8 = mybir.dt.uint8

N_CORES = 8
NT, NO = 4, 2          # token shards x outfeature shards
TOK, IN_F, OUT_F = 8192, 4096, 4096
T = TOK // NT          # 2048 tokens per core
OS = OUT_F // NO       # 2048 outfeatures per core
P = 128
NB = IN_F // P         # 32 contraction super-tiles
OB = 256               # outfeature block (psum free dim)
NOB = OS // OB         # 8
NTB = T // P           # 16 token blocks
NQ = 4                 # token-blocks shipped k-major for the head phase

CB = 8                 # super-tiles per weight-stream chunk
NCH = NB // CB         # 4 chunks per outfeature block
CBX = 4                # super-tiles per k-major x chunk
NCHX = NB // CBX       # 8 chunks for the k-major x stream

_CACHE = {}


def _build_program():
    nc = bacc.Bacc("TRN2", target_bir_lowering=False, debug=False,
                   num_devices=N_CORES)
    xq_ap = nc.dram_tensor("xq", [NCHX, P, CBX, NQ, P], BF16,
                           kind="ExternalInput").ap()
    xt_ap = nc.dram_tensor("xt", [NTB - NQ, P, NB, P], BF16,
                           kind="ExternalInput").ap()
    qs_ap = nc.dram_tensor("qs", [NOB, NCH, P, CB * OB], U8,
                           kind="ExternalInput").ap()
    sz_ap = nc.dram_tensor("sz", [NOB, NCH, CB, 2 * OB], BF16,
                           kind="ExternalInput").ap()
    br_ap = nc.dram_tensor("br", [OS], BF16, kind="ExternalInput").ap()
    y_ap = nc.dram_tensor("y", [NTB, NOB, P, OB], F32, kind="ExternalOutput").ap()

    with tile.TileContext(nc) as tc:
        with tc.tile_pool(name="resident", bufs=1) as rpool, \
             tc.tile_pool(name="wset", bufs=3) as wpool, \
             tc.tile_pool(name="qstream", bufs=2) as qpool, \
             tc.tile_pool(name="szstream", bufs=2) as szpool, \
             tc.tile_pool(name="ostream", bufs=4) as opool, \
             tc.tile_pool(name="psum", bufs=8, space="PSUM") as ppool:
            br_sb = rpool.tile([P, OS], BF16)
            nc.sync.dma_start(br_sb[:], br_ap.partition_broadcast(P))
            # zeros rhs for PE-warmup matmuls during the load phase
            wz = rpool.tile([P, OB], BF16)
            nc.gpsimd.memset(wz[:], 0.0)
            xq_sb = rpool.tile([P, NB, NQ, P], BF16)    # tb 0..3, k-major
            xt_sb = rpool.tile([P, NTB - NQ, NB, P], BF16)

            def lhsT(tb, b):
                if tb < NQ:
                    return xq_sb[:, b, tb, :]
                return xt_sb[:, tb - NQ, b, :]

            # junk psum tile for PE warmup; returns to the pool for the
            # final head group once the junk matmuls are done
            js = ppool.tile([P, OB], F32, tag="ps", name="js")
            for _ in range(2):
                nc.tensor.matmul(js[:], wz[:, :P], wz[:], start=True, stop=True)

            def dequant(wset, q_sb, sz_sb, ch):
                for l in range(CB):
                    b = ch * CB + l
                    tmp = qpool.tile([P, OB], BF16, tag="tmp")
                    nc.vector.tensor_tensor(
                        tmp[:], q_sb[:, l * OB:(l + 1) * OB],
                        sz_sb[:, l, OB:], mybir.AluOpType.subtract)
                    nc.vector.tensor_tensor(
                        wset[:, b, :], tmp[:], sz_sb[:, l, :OB],
                        mybir.AluOpType.mult)

            # --- head streams: ob0+ob1 weights and the k-major x quartet,
            # interleaved across both HWDGE rings in need-order
            wsets = [wpool.tile([P, NB, OB], BF16, tag="wset", name=f"w{o}")
                     for o in range(2)]
            for ch in range(NCH):
                for o in range(2):
                    q_sb = qpool.tile([P, CB * OB], U8, tag="q")
                    nc.sync.dma_start(q_sb[:], qs_ap[o, ch])
                    sz_sb = szpool.tile([P, CB, 2 * OB], BF16, tag="sz")
                    nc.scalar.dma_start(sz_sb[:],
                                        sz_ap[o, ch].partition_broadcast(P))
                    if ch == 0:
                        # junk matmul on arrived bytes keeps the PE p-state
                        # ramping before the first dequanted weights exist
                        nc.tensor.matmul(
                            js[:], q_sb[:, :2 * P].bitcast(BF16), wz[:],
                            start=True, stop=True)
                    dequant(wsets[o], q_sb, sz_sb, ch)
                for i in range(2):
                    cx = 2 * ch + i
                    eng = nc.sync if i == 0 else nc.scalar
                    eng.dma_start(xq_sb[:, CBX * cx:CBX * (cx + 1)], xq_ap[cx])

            # rest of the x stream (tb 4..15), striped across the rings
            for tb in range(NQ, NTB):
                eng = nc.scalar if tb % 2 else nc.sync
                eng.dma_start(xt_sb[:, tb - NQ], xt_ap[tb - NQ])

            def evac(pslice, tb, ob):
                ot = opool.tile([P, OB], F32, tag="ot")
                nc.vector.tensor_tensor(
                    ot[:], pslice, br_sb[:, ob * OB:(ob + 1) * OB],
                    mybir.AluOpType.add)
                nc.gpsimd.dma_start(y_ap[tb, ob], ot[:])

            def produce_wset(ob):
                wset = wpool.tile([P, NB, OB], BF16, tag="wset")
                for ch in range(NCH):
                    q_sb = qpool.tile([P, CB * OB], U8, tag="q")
                    nc.sync.dma_start(q_sb[:], qs_ap[ob, ch])
                    sz_sb = szpool.tile([P, CB, 2 * OB], BF16, tag="sz")
                    nc.scalar.dma_start(sz_sb[:],
                                        sz_ap[ob, ch].partition_broadcast(P))
                    dequant(wset, q_sb, sz_sb, ch)
                return wset

            # --- paired head: 4 quartets of (4 token-blocks x 2 obs) with
            # the b-loop outermost — 8 concurrent PSUM groups consume every
            # x-slice and weight tile the moment it lands
            for qd in range(NTB // NQ):
                pst = [ppool.tile([P, OB], F32, tag="ps",
                                  name=f"p{qd}_{g}") for g in range(2 * NQ)]
                for b in range(NB):
                    for g in range(2 * NQ):
                        tb, o = qd * NQ + g % NQ, g // NQ
                        nc.tensor.matmul(
                            pst[g][:], lhsT(tb, b), wsets[o][:, b, :],
                            start=(b == 0), stop=(b == NB - 1))
                for g in range(2 * NQ):
                    evac(pst[g][:], qd * NQ + g % NQ, g // NQ)

            for ob in range(2, NOB):
                wset = produce_wset(ob)
                for tb in range(NTB):
                    ps = ppool.tile([P, OB], F32, tag="ps")
                    for b in range(NB):
                        nc.tensor.matmul(
                            ps[:], lhsT(tb, b), wset[:, b, :],
                            start=(b == 0), stop=(b == NB - 1))
                    evac(ps[:], tb, ob)

    nc.compile()
    return nc


def _host_prep(x, qweight, scales, qzeros, bias):
    """Per-core input maps: layout prep only (transpose / nibble byte-split);
    dequantization (zero-subtract, scale-multiply) happens on-chip."""
    bf16 = ml_dtypes.bfloat16
    x = np.asarray(x, dtype=np.float32)
    qw = np.asarray(qweight).astype(np.int64, copy=False)
    sc = np.asarray(scales, dtype=np.float32)
    qz = np.asarray(qzeros).astype(np.int64, copy=False)
    bi = np.asarray(bias, dtype=np.float32)

    # zeros: unpack along outfeatures, +1 (pack() stored z-1)
    shifts = (np.arange(16, dtype=np.uint64) * np.uint64(4))
    zz = ((qz.astype(np.uint64)[:, :, None] >> shifts[None, None, :])
          & np.uint64(15)).reshape(qz.shape[0], -1).astype(np.float32) + 1.0

    # per-token-shard xT: tb 0..3 k-major [NCHX, P, CBX, NQ, P];
    # tb 4..15 token-major [NTB-NQ, P, NB, P]
    xq_list, xt_list = [], []
    for tc in range(NT):
        xs = x[tc * T:(tc + 1) * T]                      # [T, IN_F]
        xt = np.ascontiguousarray(xs.T).astype(bf16)     # [IN_F, T]
        xt4 = xt.reshape(NB, P, NTB, P).transpose(2, 1, 0, 3)  # [tb, p, b, t]
        xq = np.ascontiguousarray(
            xt4[:NQ].transpose(2, 1, 0, 3)               # [b, p, tb, t]
               .reshape(NCHX, CBX, P, NQ, P).transpose(0, 2, 1, 3, 4))
        xq_list.append(xq)
        xt_list.append(np.ascontiguousarray(xt4[NQ:]))

    # per-outfeature-shard weight-side tensors (shared by NT cores)
    qs_list, sz_list, br_list = [], [], []
    for oc in range(NO):
        o0 = oc * OS
        qsl = np.ascontiguousarray(qw[:, o0:o0 + OS])    # [256, OS] int64
        qbytes = qsl.view(np.uint8).reshape(IN_F // 16, OS, 8)
        qb2 = np.ascontiguousarray(qbytes.transpose(0, 2, 1)).reshape(IN_F // 2, OS)
        nib = np.empty((IN_F, OS), np.uint8)             # row k: code(k, o) << 4
        nib[0::2] = (qb2 & np.uint8(15)) << np.uint8(4)
        nib[1::2] = qb2 & np.uint8(0xF0)
        qs_t = np.ascontiguousarray(
            nib.reshape(NCH, CB, P, NOB, OB).transpose(3, 0, 2, 1, 4)
               .reshape(NOB, NCH, P, CB * OB))
        qs_list.append(qs_t)

        s16 = (sc[:, o0:o0 + OS] / 16.0).astype(bf16).reshape(NB, NOB, OB)
        z16 = (zz[:, o0:o0 + OS] * 16.0).astype(bf16).reshape(NB, NOB, OB)
        sz = np.concatenate([s16, z16], axis=-1)         # [NB, NOB, 2*OB]
        sz_t = np.ascontiguousarray(
            sz.reshape(NCH, CB, NOB, 2 * OB).transpose(2, 0, 1, 3))
        sz_list.append(sz_t)                             # [NOB, NCH, CB, 2*OB]
        br_list.append(np.ascontiguousarray(bi[o0:o0 + OS].astype(bf16)))

    in_maps = []
    for c in range(N_CORES):
        tc, oc = c // NO, c % NO
        in_maps.append({
            "xq": xq_list[tc],
            "xt": xt_list[tc],
            "qs": qs_list[oc],
            "sz": sz_list[oc],
            "br": br_list[oc],
        })
    return in_maps


def get_program():
    if "nc" not in _CACHE:
        _CACHE["nc"] = _build_program()
    return _CACHE["nc"]


def kernel(x, qweight, scales, qzeros, g_idx, bias):
    nc = get_program()
    in_maps = _host_prep(x, qweight, scales, qzeros, bias)
    res = run_bass_kernel_spmd(nc, in_maps, core_ids=list(range(N_CORES)))
    y = np.empty((TOK, OUT_F), dtype=np.float32)
    for c in range(N_CORES):
        tc, oc = c // NO, c % NO
        yt = res.results[c]["y"]                         # [NTB, NOB, P, OB]
        y[tc * T:(tc + 1) * T, oc * OS:(oc + 1) * OS] = (
            yt.transpose(0, 2, 1, 3).reshape(T, OS))
    return y


# revision 8
# speedup vs baseline: 1.0226x; 1.0226x over previous
"""GPTQ int4 quantized linear (CaiQuantLinear) on 8 Trainium2 NeuronCores.

y = x @ dequant(qweight, scales, qzeros) + bias
  x: [8192, 4096] f32, qweight: [256, 4096] int64 (16x 4-bit packed along
  infeatures), scales: [32, 4096] f32, qzeros: [32, 256] int64 (packed along
  outfeatures), g_idx = arange(4096)//128, bias: [4096] f32 -> y: [8192, 4096] f32

Sharding: 4 token-shards x 2 outfeature-shards = 8 cores. Core c handles
tokens [2048*(c//2), +2048) and outfeatures [2048*(c%2), +2048).

Device kernel (per core): weights ship as one byte per nibble with the
4-bit code in the HIGH bits (host bit-shuffle only), so dequant is two
tensor_tensor ops: (q_u8 - 16z) * (s/16), with scale/zero rows shipped
compact (262KB total) and replicated across partitions on-chip by 0-stride
partition-broadcast DMAs. Replication writes count against the ~420GB/s
DMA fabric like HBM reads, and take ~19us per weight set on one ring —
so a phase that needs a full weight set (or full token-tile) up front
stalls. The first two outfeature blocks therefore open with a b-outer
"octet": 8 concurrent PSUM groups over token-blocks 0-7 (shipped k-major)
consume each 32KB x-slice and each dequanted k-tile at 0.88us/tile, slower
than every stream arrives. By ob2 the x shard is resident and the slim
stream (1MB + replication) trickles in far ahead of each block. Matmuls
accumulate 32 k-tiles of [128,128]x[128,256] bf16 into PSUM; evacuation
adds the bias.
"""

import sys

if "/opt/trn_rl_repo" not in sys.path:
    sys.path.insert(0, "/opt/trn_rl_repo")

import numpy as np
import ml_dtypes

import concourse.bass as bass  # noqa: F401  (registers mybir types)
import concourse.mybir as mybir
import concourse.tile as tile
from concourse import bacc
from concourse.bass_utils import run_bass_kernel_spmd

BF16 = mybir.dt.bfloat16
F32 = mybir.dt.float32
U8 = mybir.dt.uint8

N_CORES = 8
NT, NO = 4, 2          # token shards x outfeature shards
TOK, IN_F, OUT_F = 8192, 4096, 4096
T = TOK // NT          # 2048 tokens per core
OS = OUT_F // NO       # 2048 outfeatures per core
P = 128
NB = IN_F // P         # 32 contraction super-tiles
OB = 256               # outfeature block (psum free dim)
NOB = OS // OB         # 8
NTB = T // P           # 16 token blocks
NQ = 8                 # token-blocks shipped k-major for the octet phases

CB = 8                 # super-tiles per weight-stream chunk
NCH = NB // CB         # 4 chunks per outfeature block
CBX = 4                # super-tiles per k-major x chunk
NCHX = NB // CBX       # 8 chunks for the k-major x stream

_CACHE = {}


def _build_program():
    nc = bacc.Bacc("TRN2", target_bir_lowering=False, debug=False,
                   num_devices=N_CORES)
    xq_ap = nc.dram_tensor("xq", [NCHX, P, CBX, NQ, P], BF16,
                           kind="ExternalInput").ap()
    xt_ap = nc.dram_tensor("xt", [NTB - NQ, P, NB, P], BF16,
                           kind="ExternalInput").ap()
    qs_ap = nc.dram_tensor("qs", [NOB, NCH, P, CB * OB], U8,
                           kind="ExternalInput").ap()
    sz_ap = nc.dram_tensor("sz", [NOB, NCH, CB, 2 * OB], BF16,
                           kind="ExternalInput").ap()
    br_ap = nc.dram_tensor("br", [OS], BF16, kind="ExternalInput").ap()
    y_ap = nc.dram_tensor("y", [NTB, NOB, P, OB], F32, kind="ExternalOutput").ap()

    with tile.TileContext(nc) as tc:
        with tc.tile_pool(name="resident", bufs=1) as rpool, \
             tc.tile_pool(name="wset", bufs=2) as wpool, \
             tc.tile_pool(name="qstream", bufs=3) as qpool, \
             tc.tile_pool(name="szstream", bufs=3) as szpool, \
             tc.tile_pool(name="ostream", bufs=6) as opool, \
             tc.tile_pool(name="psum", bufs=8, space="PSUM") as ppool:
            # bias replicate on the otherwise-idle gpsimd queue (needed
            # only by the first evacuation ~40us in)
            br_sb = rpool.tile([P, OS], BF16)
            nc.gpsimd.dma_start(br_sb[:], br_ap.partition_broadcast(P))
            # zeros rhs for PE-warmup matmuls during the load phase
            wz = rpool.tile([P, OB], BF16)
            nc.gpsimd.memset(wz[:], 0.0)
            xq_sb = rpool.tile([P, NB, NQ, P], BF16)    # tb 0..7, k-major
            xt_sb = rpool.tile([P, NTB - NQ, NB, P], BF16)

            def lhsT(tb, b):
                if tb < NQ:
                    return xq_sb[:, b, tb, :]
                return xt_sb[:, tb - NQ, b, :]

            # junk psum tile for PE warmup; rotation hands it to the last
            # octet group once the junk matmuls are done
            js = ppool.tile([P, OB], F32, tag="ps", name="js")
            for _ in range(2):
                nc.tensor.matmul(js[:], wz[:, :P], wz[:], start=True, stop=True)

            def dequant(wset, q_sb, sz_sb, ch):
                for l in range(CB):
                    b = ch * CB + l
                    tmp = qpool.tile([P, OB], BF16, tag="tmp")
                    nc.vector.tensor_tensor(
                        tmp[:], q_sb[:, l * OB:(l + 1) * OB],
                        sz_sb[:, l, OB:], mybir.AluOpType.subtract)
                    nc.vector.tensor_tensor(
                        wset[:, b, :], tmp[:], sz_sb[:, l, :OB],
                        mybir.AluOpType.mult)

            def produce_wset(ob, warm=False):
                wset = wpool.tile([P, NB, OB], BF16, tag="wset")
                for ch in range(NCH):
                    q_sb = qpool.tile([P, CB * OB], U8, tag="q")
                    nc.sync.dma_start(q_sb[:], qs_ap[ob, ch])
                    sz_sb = szpool.tile([P, CB, 2 * OB], BF16, tag="sz")
                    nc.scalar.dma_start(sz_sb[:],
                                        sz_ap[ob, ch].partition_broadcast(P))
                    if warm and ch == 0:
                        # junk matmul on arrived bytes keeps the PE p-state
                        # ramping before the first dequanted weights exist
                        nc.tensor.matmul(
                            js[:], q_sb[:, :2 * P].bitcast(BF16), wz[:],
                            start=True, stop=True)
                    dequant(wset, q_sb, sz_sb, ch)
                    if warm:
                        # k-major x chunks ride between the weight chunks
                        for i in range(2):
                            cx = 2 * ch + i
                            eng = nc.sync if i == 0 else nc.scalar
                            eng.dma_start(xq_sb[:, CBX * cx:CBX * (cx + 1)],
                                          xq_ap[cx])
                return wset

            wset0 = produce_wset(0, warm=True)
            for tb in range(NQ, NTB):
                eng = nc.scalar if tb % 2 else nc.sync
                eng.dma_start(xt_sb[:, tb - NQ], xt_ap[tb - NQ])

            def evac(pslice, tb, ob):
                ot = opool.tile([P, OB], F32, tag="ot")
                nc.vector.tensor_tensor(
                    ot[:], pslice, br_sb[:, ob * OB:(ob + 1) * OB],
                    mybir.AluOpType.add)
                nc.gpsimd.dma_start(y_ap[tb, ob], ot[:])

            def octet(ob, wset):
                pst = [ppool.tile([P, OB], F32, tag="ps",
                                  name=f"o{ob}_{g}") for g in range(NQ)]
                for b in range(NB):
                    for g in range(NQ):
                        nc.tensor.matmul(
                            pst[g][:], xq_sb[:, b, g, :], wset[:, b, :],
                            start=(b == 0), stop=(b == NB - 1))
                for g in range(NQ):
                    evac(pst[g][:], g, ob)

            def group(tb, ob, wset):
                ps = ppool.tile([P, OB], F32, tag="ps")
                for b in range(NB):
                    nc.tensor.matmul(
                        ps[:], lhsT(tb, b), wset[:, b, :],
                        start=(b == 0), stop=(b == NB - 1))
                evac(ps[:], tb, ob)

            # ob0: b-outer octet while the streams land, then token-outer
            octet(0, wset0)
            wset1 = produce_wset(1)
            for tb in range(NQ, NTB):
                group(tb, 0, wset0)
            # ob1: octet again (x quartet is resident; weights b-paced)
            octet(1, wset1)
            for tb in range(NQ, NTB):
                group(tb, 1, wset1)

            for ob in range(2, NOB):
                wset = produce_wset(ob)
                for tb in range(NTB):
                    group(tb, ob, wset)

    nc.compile()
    return nc


def _host_prep(x, qweight, scales, qzeros, bias):
    """Per-core input maps: layout prep only (transpose / nibble byte-split);
    dequantization (zero-subtract, scale-multiply) happens on-chip."""
    bf16 = ml_dtypes.bfloat16
    x = np.asarray(x, dtype=np.float32)
    qw = np.asarray(qweight).astype(np.int64, copy=False)
    sc = np.asarray(scales, dtype=np.float32)
    qz = np.asarray(qzeros).astype(np.int64, copy=False)
    bi = np.asarray(bias, dtype=np.float32)

    # zeros: unpack along outfeatures, +1 (pack() stored z-1)
    shifts = (np.arange(16, dtype=np.uint64) * np.uint64(4))
    zz = ((qz.astype(np.uint64)[:, :, None] >> shifts[None, None, :])
          & np.uint64(15)).reshape(qz.shape[0], -1).astype(np.float32) + 1.0

    # per-token-shard xT: tb 0..7 k-major [NCHX, P, CBX, NQ, P];
    # tb 8..15 token-major [NTB-NQ, P, NB, P]
    xq_list, xt_list = [], []
    for tc in range(NT):
        xs = x[tc * T:(tc + 1) * T]                      # [T, IN_F]
        xt = np.ascontiguousarray(xs.T).astype(bf16)     # [IN_F, T]
        xt4 = xt.reshape(NB, P, NTB, P).transpose(2, 1, 0, 3)  # [tb, p, b, t]
        xq = np.ascontiguousarray(
            xt4[:NQ].transpose(2, 1, 0, 3)               # [b, p, tb, t]
               .reshape(NCHX, CBX, P, NQ, P).transpose(0, 2, 1, 3, 4))
        xq_list.append(xq)
        xt_list.append(np.ascontiguousarray(xt4[NQ:]))

    # per-outfeature-shard weight-side tensors (shared by NT cores)
    qs_list, sz_list, br_list = [], [], []
    for oc in range(NO):
        o0 = oc * OS
        qsl = np.ascontiguousarray(qw[:, o0:o0 + OS])    # [256, OS] int64
        qbytes = qsl.view(np.uint8).reshape(IN_F // 16, OS, 8)
        qb2 = np.ascontiguousarray(qbytes.transpose(0, 2, 1)).reshape(IN_F // 2, OS)
        nib = np.empty((IN_F, OS), np.uint8)             # row k: code(k, o) << 4
        nib[0::2] = (qb2 & np.uint8(15)) << np.uint8(4)
        nib[1::2] = qb2 & np.uint8(0xF0)
        qs_t = np.ascontiguousarray(
            nib.reshape(NCH, CB, P, NOB, OB).transpose(3, 0, 2, 1, 4)
               .reshape(NOB, NCH, P, CB * OB))
        qs_list.append(qs_t)

        s16 = (sc[:, o0:o0 + OS] / 16.0).astype(bf16).reshape(NB, NOB, OB)
        z16 = (zz[:, o0:o0 + OS] * 16.0).astype(bf16).reshape(NB, NOB, OB)
        sz = np.concatenate([s16, z16], axis=-1)         # [NB, NOB, 2*OB]
        sz_t = np.ascontiguousarray(
            sz.reshape(NCH, CB, NOB, 2 * OB).transpose(2, 0, 1, 3))
        sz_list.append(sz_t)                             # [NOB, NCH, CB, 2*OB]
        br_list.append(np.ascontiguousarray(bi[o0:o0 + OS].astype(bf16)))

    in_maps = []
    for c in range(N_CORES):
        tc, oc = c // NO, c % NO
        in_maps.append({
            "xq": xq_list[tc],
            "xt": xt_list[tc],
            "qs": qs_list[oc],
            "sz": sz_list[oc],
            "br": br_list[oc],
        })
    return in_maps


def get_program():
    if "nc" not in _CACHE:
        _CACHE["nc"] = _build_program()
    return _CACHE["nc"]


def kernel(x, qweight, scales, qzeros, g_idx, bias):
    nc = get_program()
    in_maps = _host_prep(x, qweight, scales, qzeros, bias)
    res = run_bass_kernel_spmd(nc, in_maps, core_ids=list(range(N_CORES)))
    y = np.empty((TOK, OUT_F), dtype=np.float32)
    for c in range(N_CORES):
        tc, oc = c // NO, c % NO
        yt = res.results[c]["y"]                         # [NTB, NOB, P, OB]
        y[tc * T:(tc + 1) * T, oc * OS:(oc + 1) * OS] = (
            yt.transpose(0, 2, 1, 3).reshape(T, OS))
    return y
